# revision 1
# baseline (speedup 1.0000x reference)
import sys
sys.path.insert(0, '/opt/trn_rl_repo')
import numpy as np
import concourse.bass as bass
import concourse.bacc as bacc
import concourse.mybir as mybir
from concourse.tile import TileContext
from concourse import bass_utils

f32, bf16 = mybir.dt.float32, mybir.dt.bfloat16
Alu = mybir.AluOpType

B, C, H, W = 16, 3, 1024, 1024
NPIX = H * W            # 1048576 = 2^20
NS = 6                  # slices per core (48 / 8)
P = 128
FD = NPIX // P          # 8192
CH = 1024               # chunk free-dim
NCH = FD // CH          # 8
INV_NPIX = 1.0 / NPIX


def build_program():
    nc = bacc.Bacc()
    x6 = nc.dram_tensor("x6", [NS, NPIX], f32, kind="ExternalInput")
    msk = nc.dram_tensor("msk", [P, 256], f32, kind="ExternalInput")  # 1 where k%16==15
    y6 = nc.dram_tensor("y6", [NS, NPIX], f32, kind="ExternalOutput")

    with TileContext(nc) as tc:
        with tc.tile_pool(name="sb", bufs=1) as sb, \
             tc.tile_pool(name="ps", bufs=1, space="PSUM") as ps:
            mskt = sb.tile([P, 256], f32)
            nc.sync.dma_start(mskt, msk[:, :])
            onesc = sb.tile([1, 128], f32)
            nc.vector.memset(onesc, 1.0)

            for s in range(NS):
                xs = x6[s, :].rearrange("(p f) -> p f", p=P)
                v = sb.tile([P, FD], f32, tag="v")
                nc.sync.dma_start(v, xs)
                nc.vector.tensor_scalar(v, v, 255.0, None, Alu.mult)

                Sps = ps.tile([17, 16], f32, tag="Sps")
                for c in range(NCH):
                    vc = v[:, c * CH:(c + 1) * CH]
                    # w = v mod 16 via binary peel (exact fp32)
                    b = sb.tile([P, CH], f32, tag="b")
                    w = sb.tile([P, CH], f32, tag="w")
                    nc.vector.tensor_scalar(b, vc, 128.0, None, Alu.is_ge)
                    nc.vector.scalar_tensor_tensor(w, b, -128.0, vc, Alu.mult, Alu.add)
                    for lev in (64.0, 32.0, 16.0):
                        nc.vector.tensor_scalar(b, w, lev, None, Alu.is_ge)
                        nc.vector.scalar_tensor_tensor(w, b, -lev, w, Alu.mult, Alu.add)
                    # step planes, laid (P, CH, nslots) so lhsT views are contiguous
                    sa = sb.tile([P, CH, 17], bf16, tag="sa")
                    sbp = sb.tile([P, CH, 16], bf16, tag="sbp")
                    for h in range(17):
                        nc.vector.tensor_scalar(sa[:, :, h], vc, 16.0 * h, None, Alu.is_ge)
                    for l in range(16):
                        nc.vector.tensor_scalar(sbp[:, :, l], w, float(l), None, Alu.is_ge)
                    for j in range(CH):
                        nc.tensor.matmul(Sps, sa[:, j, :], sbp[:, j, :],
                                         start=(c == 0 and j == 0),
                                         stop=(c == NCH - 1 and j == CH - 1))

                # S (17x16) -> flat (1x288, zero padded) -> replicate to 128 partitions
                Ssb = sb.tile([17, 16], f32, tag="Ssb")
                nc.vector.tensor_copy(Ssb, Sps)
                flat = sb.tile([1, 288], f32, tag="flat")
                nc.vector.memset(flat, 0.0)
                nc.sync.dma_start(flat[0:1, 0:272], Ssb[:, :])
                repp = ps.tile([128, 288], f32, tag="repp")
                nc.tensor.matmul(repp, onesc, flat, start=True, stop=True)
                Sr = sb.tile([P, 288], f32, tag="Sr")
                nc.vector.tensor_copy(Sr, repp)

                # hist weights W[k] = (S[k]-S[k+1]-S[k+16]+S[k+17])/NPIX, k=0..255
                # patch k%16==15: += (S[k+1]-S[k+17])  [restores the wrapped column]
                Wf = sb.tile([P, 256], f32, tag="Wf")
                t2 = sb.tile([P, 256], f32, tag="t2")
                nc.vector.tensor_tensor(Wf, Sr[:, 0:256], Sr[:, 16:272], Alu.subtract)
                nc.vector.tensor_tensor(t2, Sr[:, 1:257], Sr[:, 17:273], Alu.subtract)
                nc.vector.tensor_tensor(Wf, Wf, t2, Alu.subtract)
                nc.vector.tensor_tensor(t2, t2, mskt, Alu.mult)
                nc.vector.tensor_tensor(Wf, Wf, t2, Alu.add)
                nc.vector.tensor_scalar(Wf, Wf, INV_NPIX, None, Alu.mult)

                # remap: y = sum_k W[k] * (v >= k)
                y = sb.tile([P, FD], f32, tag="y")
                t = sb.tile([P, FD], f32, tag="t")
                nc.vector.tensor_scalar(y, v, 0.0, Wf[:, 0:1], Alu.is_ge, Alu.mult)
                for k in range(1, 256):
                    nc.vector.tensor_scalar(t, v, float(k), Wf[:, k:k + 1], Alu.is_ge, Alu.mult)
                    nc.vector.tensor_tensor(y, y, t, Alu.add)
                nc.sync.dma_start(y6[s, :].rearrange("(p f) -> p f", p=P), y)
    nc.finalize()
    return nc


_mask = None


def kernel(x):
    global _mask
    x = np.asarray(x, dtype=np.float32)
    xr = x.reshape(48, NPIX)
    if _mask is None:
        m = np.zeros((P, 256), dtype=np.float32)
        m[:, 15::16] = 1.0
        _mask = m
    nc = build_program()
    in_maps = [{"x6": np.ascontiguousarray(xr[i * NS:(i + 1) * NS]), "msk": _mask}
               for i in range(8)]
    res = bass_utils.run_bass_kernel_spmd(nc, in_maps, core_ids=list(range(8)))
    y = np.concatenate([r["y6"].reshape(NS, NPIX) for r in res.results], axis=0)
    return y.reshape(B, C, H, W).astype(np.float32)

